# revision 3
# baseline (speedup 1.0000x reference)
"""JetMoE MoA kernel for 8 Trainium2 NeuronCores.

Strategy (expert-parallel, per the sharding hint):
  - Host computes the router (logits / top-2 / softmax gates) and uses it to
    DISPATCH: tokens routed to expert e are gathered into a compact, padded,
    transposed buffer xeT for core e (this is the "all-to-all" done at input
    staging time, since kernel() receives full inputs on the host).
  - Per-expert weights are algebraically combined on the host:
        w_comb[e] = w_out[e] @ w_in[e]            (f64, cast to f32)
    so each core runs ONE dense GEMM over its dispatched tokens:
        z_e = (xe @ w_comb[e].T) * gates_e[:, None]
    computed on the PE array in float32r (full-rate fp32 mode).
  - Host combines: out[t] = z[slot0(t)] + z[slot1(t)] + bias  (index-add),
    and returns (out, logits) exactly like the reference.

Device kernel (per core, SPMD on cores 0-7):
    aT [K=2048, C]   gathered tokens, transposed (contraction-major)
    bT [K=2048, N=2048]  w_comb[e].T  (contraction-major)
    g  [C]           gate weight per dispatched token (0 for padding)
    z  [C, N]        output rows, gate-scaled

GEMM structure: bT is fully resident in SBUF (16 k-slabs); the m-loop streams
aT m-slabs; inner loops are k-outer/n-inner so each stationary (lhsT) k-tile
is reused across all 4 moving n-chunks; 4 PSUM accumulation groups run per
m-tile; DVE applies the per-row gate on PSUM eviction.
"""

import numpy as np

NUM_EXPERTS = 8
TOP_K = 2
D = 2048  # hidden size (contraction dim K)
P = 128
NF = 512  # moving free-dim chunk (one PSUM bank, fp32)

_compiled_cache = {}


def _build_gemm(C):
    import concourse.mybir as mybir
    from concourse import bacc
    from concourse.tile import TileContext

    dt_mm = mybir.dt.float32r
    K = D
    N = D
    KO = K // P
    MT = C // P
    NCH = N // NF

    nc = bacc.Bacc(None, target_bir_lowering=False)
    aT = nc.declare_dram_parameter("aT", [K, C], dt_mm, isOutput=False)
    bT = nc.declare_dram_parameter("bT", [K, N], dt_mm, isOutput=False)
    g = nc.declare_dram_parameter("g", [C], mybir.dt.float32, isOutput=False)
    z = nc.declare_dram_parameter("z", [C, N], mybir.dt.float32, isOutput=True)

    with TileContext(nc) as tc:
        with (
            tc.tile_pool(name="bt", bufs=1) as bt_pool,
            tc.tile_pool(name="a", bufs=3) as a_pool,
            tc.tile_pool(name="gp", bufs=1) as g_pool,
            tc.tile_pool(name="out", bufs=3) as out_pool,
            tc.tile_pool(name="ps", bufs=2, space="PSUM") as ps_pool,
        ):
            bt_slabs = []
            for ko in range(KO):
                slab = bt_pool.tile(
                    [P, N], dt_mm, name=f"bt{ko}", tag=f"bt{ko}"
                )
                nc.sync.dma_start(out=slab[:], in_=bT[ko * P : (ko + 1) * P, :])
                bt_slabs.append(slab)
            g_sb = g_pool.tile([P, MT], mybir.dt.float32)
            nc.sync.dma_start(out=g_sb[:], in_=g.rearrange("(mo mi) -> mi mo", mi=P))

            for m in range(MT):
                a_sb = a_pool.tile([P, KO, P], dt_mm)
                nc.sync.dma_start(
                    out=a_sb[:],
                    in_=aT[:, m * P : (m + 1) * P].rearrange(
                        "(ko ki) m -> ki ko m", ki=P
                    ),
                )
                out_sb = out_pool.tile([P, N], mybir.dt.float32)
                pss = [
                    ps_pool.tile([P, NF], mybir.dt.float32, name=f"ps{n}", tag=f"ps{n}")
                    for n in range(NCH)
                ]
                for ko in range(KO):
                    for n in range(NCH):
                        nc.tensor.matmul(
                            pss[n][:],
                            a_sb[:, ko, :],
                            bt_slabs[ko][:, n * NF : (n + 1) * NF],
                            start=(ko == 0),
                            stop=(ko == KO - 1),
                        )
                for n in range(NCH):
                    nc.vector.tensor_scalar_mul(
                        out_sb[:, n * NF : (n + 1) * NF],
                        pss[n][:],
                        g_sb[:, m : m + 1],
                    )
                nc.sync.dma_start(out=z[m * P : (m + 1) * P, :], in_=out_sb[:])
    nc.finalize()
    return nc


def _get_compiled(C):
    if C not in _compiled_cache:
        _compiled_cache[C] = _build_gemm(C)
    return _compiled_cache[C]


def kernel(layer_input, w_gate, w_in, w_out, bias):
    from concourse.bass_utils import run_bass_kernel_spmd

    B, L, Din = layer_input.shape
    assert Din == D
    E = NUM_EXPERTS
    x = np.ascontiguousarray(layer_input.reshape(-1, D), dtype=np.float32)
    T = x.shape[0]

    # ---- router on host (f64 accumulate, f32 result like the reference) ----
    logits = (x.astype(np.float64) @ w_gate.astype(np.float64).T).astype(np.float32)
    top_idx = np.argsort(-logits, axis=1, kind="stable")[:, :TOP_K]  # [T, K]
    top_logits = np.take_along_axis(logits, top_idx, axis=1)  # [T, K] f32
    mx = top_logits.max(axis=1, keepdims=True)
    ex = np.exp(top_logits - mx, dtype=np.float32)
    gates = ex / ex.sum(axis=1, keepdims=True)  # [T, K] f32

    # ---- dispatch: gather token-slots per expert ----
    e_flat = top_idx.reshape(-1)  # [T*K]
    order = np.argsort(e_flat, kind="stable")  # token-slots grouped by expert
    counts = np.bincount(e_flat, minlength=E)
    C = max(P, int(np.ceil(counts.max() / P)) * P)

    gate_flat = gates.reshape(-1)
    tok_of_slot = order // TOP_K  # token index for each sorted slot
    starts = np.zeros(E + 1, dtype=np.int64)
    np.cumsum(counts, out=starts[1:])

    nc = _get_compiled(C)

    in_maps = []
    # positions of each token-slot within the padded per-expert buffers
    gpos = np.empty(T * TOP_K, dtype=np.int64)
    for e in range(E):
        sl = order[starts[e] : starts[e + 1]]
        n_e = sl.shape[0]
        toks = tok_of_slot[starts[e] : starts[e + 1]]
        xe = x[toks]  # [n_e, D]
        xeT = np.zeros((D, C), dtype=np.float32)
        xeT[:, :n_e] = xe.T
        g_e = np.zeros(C, dtype=np.float32)
        g_e[:n_e] = gate_flat[sl]
        # combined expert weight, transposed to contraction-major
        w_comb = w_out[e].astype(np.float64) @ w_in[e].astype(np.float64)
        bT = np.ascontiguousarray(w_comb.T, dtype=np.float32)
        in_maps.append({"aT": xeT, "bT": bT, "g": g_e})
        gpos[sl] = e * C + np.arange(n_e)

    res = run_bass_kernel_spmd(nc, in_maps, core_ids=list(range(E)))
    z_cat = np.concatenate([res.results[e]["z"] for e in range(E)], axis=0)

    # ---- combine on host: index-add over the TOP_K slots + bias ----
    gp = gpos.reshape(T, TOP_K)
    out = z_cat[gp[:, 0]] + z_cat[gp[:, 1]] + bias.astype(np.float32)[None, :]
    return out.reshape(B, L, D).astype(np.float32), logits


# revision 4
# speedup vs baseline: 1.0910x; 1.0910x over previous
"""JetMoE MoA kernel for 8 Trainium2 NeuronCores.

Strategy (expert-parallel, per the sharding hint):
  - Host computes the router (logits / top-2 / softmax gates) and uses it to
    DISPATCH: tokens routed to expert e are gathered into a compact, padded,
    PE-tiled buffer for core e (the "all-to-all" done at input staging time,
    since kernel() receives full inputs on the host).
  - Per-expert weights are algebraically combined on the host:
        w_comb[e] = w_out[e] @ w_in[e]            (f64, cast to f32)
    so each core runs ONE dense GEMM over its dispatched tokens:
        z_e = (xe @ w_comb[e].T) * gates_e[:, None]
    computed on the PE array in float32r (full-rate fp32 mode).
  - Host combines: out[t] = z[slot0(t)] + z[slot1(t)] + bias  (index-add),
    and returns (out, logits) exactly like the reference.

Device kernel (per core, SPMD on cores 0-7):
    a  [MT, P, KO*P] gathered tokens, pre-tiled: a[m, ki, ko*P+j] is token
                     (m*P+j)'s feature (ko*P+ki)  -> contiguous slab DMAs
    bT [K=2048, N=2048]  w_comb[e].T  (contraction-major)
    g  [C]           gate weight per dispatched token (0 for padding)
    z  [C, N]        output rows, gate-scaled

GEMM structure: bT is fully resident in SBUF (16 k-slabs with per-slab deps);
a m-slabs are double-buffered with the first two issued ahead of bT so the PE
starts ~5us in. m-tiles 0..1 are fused k-outer across all 8 PSUM banks to give
the PE 2x work per arriving bT slab during the initial weight stream; the
remaining m-tiles run k-outer/n-inner (stationary reuse) gap-free. DVE applies
the per-row gate on PSUM eviction.
"""

import numpy as np

NUM_EXPERTS = 8
TOP_K = 2
D = 2048  # hidden size (contraction dim K)
P = 128
NF = 512  # moving free-dim chunk (one PSUM bank, fp32)
KO = D // P

_compiled_cache = {}


def _build_gemm(C):
    import concourse.mybir as mybir
    from concourse import bacc
    from concourse.tile import TileContext

    dt_mm = mybir.dt.float32r
    N = D
    MT = C // P
    NCH = N // NF

    nc = bacc.Bacc(None, target_bir_lowering=False)
    a = nc.declare_dram_parameter("a", [MT, P, KO * P], dt_mm, isOutput=False)
    bT = nc.declare_dram_parameter("bT", [D, N], dt_mm, isOutput=False)
    g = nc.declare_dram_parameter("g", [C], mybir.dt.float32, isOutput=False)
    z = nc.declare_dram_parameter("z", [C, N], mybir.dt.float32, isOutput=True)

    with TileContext(nc) as tc:
        with (
            tc.tile_pool(name="bt", bufs=1) as bt_pool,
            tc.tile_pool(name="ap", bufs=3) as a_pool,
            tc.tile_pool(name="gp", bufs=1) as g_pool,
            tc.tile_pool(name="out", bufs=3) as out_pool,
            tc.tile_pool(name="ps", bufs=2, space="PSUM") as ps_pool,
        ):
            a_tiles = {}

            def load_a(m):
                t = a_pool.tile([P, KO * P], dt_mm, name=f"a_sb{m}", tag="a_sb")
                nc.sync.dma_start(out=t[:], in_=a[m])
                a_tiles[m] = t

            # first two token slabs ahead of the weight stream
            load_a(0)
            if MT > 1:
                load_a(1)

            bt_slabs = []
            for ko in range(KO):
                slab = bt_pool.tile([P, N], dt_mm, name=f"bt{ko}", tag=f"bt{ko}")
                nc.sync.dma_start(out=slab[:], in_=bT[ko * P : (ko + 1) * P, :])
                bt_slabs.append(slab)

            g_sb = g_pool.tile([P, MT], mybir.dt.float32)
            nc.sync.dma_start(out=g_sb[:], in_=g.rearrange("(mo mi) -> mi mo", mi=P))

            def psum_group(m):
                return [
                    ps_pool.tile(
                        [P, NF], mybir.dt.float32, name=f"ps{m}_{n}", tag=f"ps{n}"
                    )
                    for n in range(NCH)
                ]

            def mm(pss, m, ko, n):
                nc.tensor.matmul(
                    pss[n][:],
                    a_tiles[m][:, ko * P : (ko + 1) * P],
                    bt_slabs[ko][:, n * NF : (n + 1) * NF],
                    start=(ko == 0),
                    stop=(ko == KO - 1),
                )

            def evict(pss, m):
                out_sb = out_pool.tile(
                    [P, N], mybir.dt.float32, name=f"out_sb{m}", tag="out_sb"
                )
                for n in range(NCH):
                    nc.vector.tensor_scalar_mul(
                        out_sb[:, n * NF : (n + 1) * NF], pss[n][:], g_sb[:, m : m + 1]
                    )
                nc.sync.dma_start(out=z[m * P : (m + 1) * P, :], in_=out_sb[:])
                a_tiles.pop(m)

            head = list(range(min(2, MT)))  # fused warm-up m-tiles
            pss_head = [psum_group(m) for m in head]
            for ko in range(KO):
                for i, m in enumerate(head):
                    for n in range(NCH):
                        mm(pss_head[i], m, ko, n)
            for i, m in enumerate(head):
                evict(pss_head[i], m)

            for m in range(len(head), MT):
                if m not in a_tiles:
                    load_a(m)
                if m + 1 < MT and (m + 1) not in a_tiles:
                    load_a(m + 1)
                pss = psum_group(m)
                for ko in range(KO):
                    for n in range(NCH):
                        mm(pss, m, ko, n)
                evict(pss, m)
    nc.finalize()
    return nc


def _get_compiled(C):
    if C not in _compiled_cache:
        _compiled_cache[C] = _build_gemm(C)
    return _compiled_cache[C]


def kernel(layer_input, w_gate, w_in, w_out, bias):
    from concourse.bass_utils import run_bass_kernel_spmd

    B, L, Din = layer_input.shape
    assert Din == D
    E = NUM_EXPERTS
    x = np.ascontiguousarray(layer_input.reshape(-1, D), dtype=np.float32)
    T = x.shape[0]

    # ---- router on host (f64 accumulate, f32 result like the reference) ----
    logits = (x.astype(np.float64) @ w_gate.astype(np.float64).T).astype(np.float32)
    top_idx = np.argsort(-logits, axis=1, kind="stable")[:, :TOP_K]  # [T, K]
    top_logits = np.take_along_axis(logits, top_idx, axis=1)  # [T, K] f32
    mx = top_logits.max(axis=1, keepdims=True)
    ex = np.exp(top_logits - mx, dtype=np.float32)
    gates = ex / ex.sum(axis=1, keepdims=True)  # [T, K] f32

    # ---- dispatch: gather token-slots per expert ----
    e_flat = top_idx.reshape(-1)  # [T*K]
    order = np.argsort(e_flat, kind="stable")  # token-slots grouped by expert
    counts = np.bincount(e_flat, minlength=E)
    C = max(P, int(np.ceil(counts.max() / P)) * P)
    MT = C // P

    gate_flat = gates.reshape(-1)
    tok_of_slot = order // TOP_K  # token index for each sorted slot
    starts = np.zeros(E + 1, dtype=np.int64)
    np.cumsum(counts, out=starts[1:])

    nc = _get_compiled(C)

    in_maps = []
    # positions of each token-slot within the padded per-expert buffers
    gpos = np.empty(T * TOP_K, dtype=np.int64)
    for e in range(E):
        sl = order[starts[e] : starts[e + 1]]
        n_e = sl.shape[0]
        toks = tok_of_slot[starts[e] : starts[e + 1]]
        xe = np.zeros((C, D), dtype=np.float32)
        xe[:n_e] = x[toks]
        # PE-tiled layout: a[m, ki, ko*P+j] = xe[m*P+j, ko*P+ki]
        a5 = np.ascontiguousarray(
            xe.reshape(MT, P, KO, P).transpose(0, 3, 2, 1).reshape(MT, P, KO * P)
        )
        g_e = np.zeros(C, dtype=np.float32)
        g_e[:n_e] = gate_flat[sl]
        # combined expert weight, transposed to contraction-major
        w_comb = w_out[e].astype(np.float64) @ w_in[e].astype(np.float64)
        bT = np.ascontiguousarray(w_comb.T, dtype=np.float32)
        in_maps.append({"a": a5, "bT": bT, "g": g_e})
        gpos[sl] = e * C + np.arange(n_e)

    res = run_bass_kernel_spmd(nc, in_maps, core_ids=list(range(E)))
    z_cat = np.concatenate([res.results[e]["z"] for e in range(E)], axis=0)

    # ---- combine on host: index-add over the TOP_K slots + bias ----
    gp = gpos.reshape(T, TOP_K)
    out = z_cat[gp[:, 0]] + z_cat[gp[:, 1]] + bias.astype(np.float32)[None, :]
    return out.reshape(B, L, D).astype(np.float32), logits
